# revision 13
# baseline (speedup 1.0000x reference)
"""Trainium2 Bass kernel for nn_Encoder (embedding lookup + GRU over T=512 steps).

Strategy (8 NeuronCores, data-parallel over batch: 128 -> 16 per core):
  Phase A  (setup): DMA weights (bf16), identities, per-core masks, token indices.
  Phase B  (precompute, parallel): indirect-DMA gather of embedding rows,
           PE-transpose to get emb^T, then PE matmuls compute the x-contributions
           XgT = (W_gx^T @ emb^T) + b_g and XcT = (W_cx^T @ emb^T) + b_c for all
           timesteps, stored transposed ([gate_dim on partitions, (t,b) free]) in
           SBUF as bf16.
  Phase C  (recurrence, 512 serial steps): per step, PSUM is preloaded with
           XgT[t]/XcT[t] via identity matmuls (off the critical chain, one step
           ahead), then gates^T += W_gh^T @ h^T, sigmoid on ScalarE, r*h on
           VectorE, candidate matmul, tanh, and the masked blend
           h' = c + max(u, invalid)*(h - c); y[t] = m*h'. The hidden state
           stays transposed [u on partitions, batch free] so every elementwise
           op uses all 128 lanes.

kernel(**inputs) takes full unsharded inputs and returns (outputs, h_final)
matching the reference (TF GRUCell semantics with dynamic_rnn length masking).
"""
import numpy as np
import ml_dtypes

import concourse.bass as bass
from concourse import bacc
import concourse.mybir as mybir
import concourse.tile as tile
from concourse.bass_utils import run_bass_kernel_spmd

BF = ml_dtypes.bfloat16
bf16 = mybir.dt.bfloat16
f32 = mybir.dt.float32
i32 = mybir.dt.int32

T, B, V, E, U = 512, 128, 50000, 256, 256
NCORES = 8
BS = B // NCORES          # 16 sequences per core
SLAB = 64                 # steps buffered per output DMA


def build_nc(t_steps=T):
    assert t_steps % 32 == 0
    ntok = t_steps * BS               # tokens per core
    ngath = ntok // 128               # 128-row gather tiles
    ngrp = ntok // 512                # 512-token matmul groups
    nslab = (t_steps + SLAB - 1) // SLAB

    nc = bacc.Bacc("TRN2", target_bir_lowering=False, debug=False,
                   num_devices=NCORES)

    emb_d = nc.dram_tensor("emb", [V, E], f32, kind="ExternalInput")
    idx_d = nc.dram_tensor("idx", [128, ngath], i32, kind="ExternalInput")
    wgh_d = nc.dram_tensor("wgh", [128, 2, 2 * U], bf16, kind="ExternalInput")
    wgx_d = nc.dram_tensor("wgx", [128, 2, 2 * U], bf16, kind="ExternalInput")
    wch_d = nc.dram_tensor("wch", [128, 2, U], bf16, kind="ExternalInput")
    wcx_d = nc.dram_tensor("wcx", [128, 2, U], bf16, kind="ExternalInput")
    bg_d = nc.dram_tensor("bg", [128, 4], f32, kind="ExternalInput")
    bc_d = nc.dram_tensor("bc", [128, 2], f32, kind="ExternalInput")
    idf_d = nc.dram_tensor("idf", [128, 128], f32, kind="ExternalInput")
    idb_d = nc.dram_tensor("idb", [128, 128], bf16, kind="ExternalInput")
    invm_d = nc.dram_tensor("invm", [1, t_steps * 32], bf16, kind="ExternalInput")
    msk_d = nc.dram_tensor("msk", [1, t_steps * 32], bf16, kind="ExternalInput")
    y_d = nc.dram_tensor("y", [t_steps, BS, U], f32, kind="ExternalOutput")
    hf_d = nc.dram_tensor("hfin", [BS, U], f32, kind="ExternalOutput")

    # 3-D DMA views (p, t, b) per u-chunk k — DMA APs allow at most 3 dims
    yviews = [y_d[:, :, k * 128:(k + 1) * 128].rearrange("t b p -> p t b")
              for k in range(2)]
    hviews = [hf_d[:, k * 128:(k + 1) * 128].rearrange("b p -> p b")
              for k in range(2)]

    Sig = mybir.ActivationFunctionType.Sigmoid
    Tanh = mybir.ActivationFunctionType.Tanh
    Alu = mybir.AluOpType

    with tile.TileContext(nc) as tc:
        with tc.tile_pool(name="const", bufs=1) as cp:
            idx_sb = cp.tile([128, ngath], i32)
            nc.sync.dma_start(out=idx_sb[:], in_=idx_d[:])
            wgh_sb = cp.tile([128, 2, 2 * U], bf16)
            nc.sync.dma_start(out=wgh_sb[:], in_=wgh_d[:])
            wgx_sb = cp.tile([128, 2, 2 * U], bf16)
            nc.sync.dma_start(out=wgx_sb[:], in_=wgx_d[:])
            wch_sb = cp.tile([128, 2, U], bf16)
            nc.sync.dma_start(out=wch_sb[:], in_=wch_d[:])
            wcx_sb = cp.tile([128, 2, U], bf16)
            nc.sync.dma_start(out=wcx_sb[:], in_=wcx_d[:])
            bg_sb = cp.tile([128, 4], f32)
            nc.sync.dma_start(out=bg_sb[:], in_=bg_d[:])
            bc_sb = cp.tile([128, 2], f32)
            nc.sync.dma_start(out=bc_sb[:], in_=bc_d[:])
            idf_sb = cp.tile([128, 128], f32)
            nc.sync.dma_start(out=idf_sb[:], in_=idf_d[:])
            idb_sb = cp.tile([128, 128], bf16)
            nc.sync.dma_start(out=idb_sb[:], in_=idb_d[:])
            invm_sb = cp.tile([128, t_steps * 32], bf16)
            nc.sync.dma_start(out=invm_sb[:],
                              in_=invm_d[:].to_broadcast((128, t_steps * 32)))
            msk_sb = cp.tile([128, t_steps * 32], bf16)
            nc.sync.dma_start(out=msk_sb[:],
                              in_=msk_d[:].to_broadcast((128, t_steps * 32)))
            h0_bf = cp.tile([128, 32], bf16)
            nc.vector.memset(h0_bf[:], 0.0)

            xgT = cp.tile([128, 4, ntok], bf16)
            xcT = cp.tile([128, 2, ntok], bf16)

            # -------- Phases B+C interleaved: precompute group g feeds
            # recurrence steps [32g, 32g+32); groups are emitted two windows
            # ahead so gather/transpose/x-matmuls hide under the recurrence.
            with tc.tile_pool(name="gath", bufs=4) as gp, \
                 tc.tile_pool(name="embt", bufs=2) as ep, \
                 tc.tile_pool(name="trps", bufs=1, space="PSUM") as trps, \
                 tc.tile_pool(name="xps", bufs=1, space="PSUM") as xps, \
                 tc.tile_pool(name="psrp", bufs=2, space="PSUM") as prp, \
                 tc.tile_pool(name="psup", bufs=2, space="PSUM") as pup, \
                 tc.tile_pool(name="pscp", bufs=2, space="PSUM") as pcp, \
                 tc.tile_pool(name="work", bufs=2) as wp, \
                 tc.tile_pool(name="hbuf", bufs=2) as hp, \
                 tc.tile_pool(name="yslab", bufs=2) as yp:

                def group_items(grp):
                    """Thunks for group grp's work, to be spread across the
                    preceding step window (keeps the in-order PE stream free
                    of long precompute bursts)."""
                    items = []
                    embT = ep.tile([128, 2, 512], bf16, tag="embT",
                                   name=f"embT{grp}")

                    def gather(j):
                        def go():
                            k = grp * 4 + j
                            et = gp.tile([128, E], f32, tag=f"et{j % 2}",
                                         name=f"et{grp}_{j}")
                            nc.gpsimd.indirect_dma_start(
                                out=et[:], out_offset=None, in_=emb_d[:],
                                in_offset=bass.IndirectOffsetOnAxis(
                                    ap=idx_sb[:, k:k + 1], axis=0))
                            return et
                        return go

                    ets = {}

                    def transpose(j, e):
                        def go():
                            et = ets[j]
                            tp = trps.tile([128, 128], f32, tag="tp",
                                           name=f"tp{grp}_{j}_{e}")
                            nc.tensor.transpose(
                                out=tp[:], in_=et[:, e * 128:(e + 1) * 128],
                                identity=idf_sb[:])
                            dst = embT[:, e, j * 128:(j + 1) * 128]
                            if (j + e) % 2 == 0:
                                nc.vector.tensor_copy(out=dst, in_=tp[:])
                            else:
                                nc.scalar.copy(out=dst, in_=tp[:])
                        return go

                    def gather_then(j):
                        g = gather(j)

                        def go():
                            ets[j] = g()
                        return go

                    def xg_mm(gt):
                        def go():
                            ps = xps.tile([128, 512], f32, tag="xps",
                                          name=f"xg{grp}_{gt}")
                            for e in range(2):
                                nc.tensor.matmul(
                                    out=ps[:],
                                    lhsT=wgx_sb[:, e, gt * 128:(gt + 1) * 128],
                                    rhs=embT[:, e, :],
                                    start=(e == 0), stop=(e == 1))
                            nc.vector.tensor_scalar(
                                out=xgT[:, gt, grp * 512:(grp + 1) * 512],
                                in0=ps[:], scalar1=bg_sb[:, gt:gt + 1],
                                scalar2=None, op0=Alu.add)
                        return go

                    def xc_mm(ct):
                        def go():
                            ps = xps.tile([128, 512], f32, tag="xps",
                                          name=f"xc{grp}_{ct}")
                            for e in range(2):
                                nc.tensor.matmul(
                                    out=ps[:],
                                    lhsT=wcx_sb[:, e, ct * 128:(ct + 1) * 128],
                                    rhs=embT[:, e, :],
                                    start=(e == 0), stop=(e == 1))
                            nc.vector.tensor_scalar(
                                out=xcT[:, ct, grp * 512:(grp + 1) * 512],
                                in0=ps[:], scalar1=bc_sb[:, ct:ct + 1],
                                scalar2=None, op0=Alu.add)
                        return go

                    for j in range(4):
                        items.append(gather_then(j))
                        items.append(transpose(j, 0))
                        items.append(transpose(j, 1))
                    for gt in range(4):
                        items.append(xg_mm(gt))
                    for ct in range(2):
                        items.append(xc_mm(ct))
                    return items

                def emit_group(grp):
                    for it in group_items(grp):
                        it()

                def preload(t):
                    """psum tiles for step t preloaded with XgT[t]/XcT[t]:
                    r and u gates in separate banks so sigmoid(r) can start
                    as soon as the r-matmuls finish."""
                    pr = prp.tile([128, 32], f32, tag="psr", name=f"psr{t}")
                    pu = pup.tile([128, 32], f32, tag="psu", name=f"psu{t}")
                    pc = pcp.tile([128, 32], f32, tag="psc", name=f"psc{t}")
                    nc.tensor.matmul(
                        out=pr[:], lhsT=idb_sb[:],
                        rhs=xgT[:, 0:2, t * 16:(t + 1) * 16],
                        start=True, stop=False, skip_group_check=True)
                    nc.tensor.matmul(
                        out=pu[:], lhsT=idb_sb[:],
                        rhs=xgT[:, 2:4, t * 16:(t + 1) * 16],
                        start=True, stop=False, skip_group_check=True)
                    nc.tensor.matmul(
                        out=pc[:], lhsT=idb_sb[:],
                        rhs=xcT[:, 0:2, t * 16:(t + 1) * 16],
                        start=True, stop=False, skip_group_check=True)
                    return pr, pu, pc

                state = {}

                def emit_step(t, item=None):
                    pr, pu, pc = state["pr"], state["pu"], state["pc"]
                    h_prev = state["h"]
                    # h'(t-1) = wc + a; feed both into the gate matmuls
                    # instead of waiting for the elementwise add (one fewer
                    # chain stage). r-gates first so sigmoid(r) fires early.
                    parts = [state["wc"], state["a"]]
                    for gt in range(2):
                        for e in range(2):
                            for pi, hpart in enumerate(parts):
                                nc.tensor.matmul(
                                    out=pr[:, gt * 16:(gt + 1) * 16],
                                    lhsT=wgh_sb[:, e, gt * 128:(gt + 1) * 128],
                                    rhs=hpart[:, e * 16:(e + 1) * 16],
                                    start=False,
                                    stop=(gt == 1 and e == 1 and pi == 1),
                                    skip_group_check=True)
                    for gt in range(2):
                        for e in range(2):
                            for pi, hpart in enumerate(parts):
                                nc.tensor.matmul(
                                    out=pu[:, gt * 16:(gt + 1) * 16],
                                    lhsT=wgh_sb[:, e, (2 + gt) * 128:(3 + gt) * 128],
                                    rhs=hpart[:, e * 16:(e + 1) * 16],
                                    start=False,
                                    stop=(gt == 1 and e == 1 and pi == 1),
                                    skip_group_check=True)
                    if item is not None:
                        item()   # PE idles here until rh is ready
                    r_sb = wp.tile([128, 32], f32, tag="r")
                    nc.scalar.activation(out=r_sb[:], in_=pr[:], func=Sig)
                    u_sb = wp.tile([128, 32], f32, tag="u")
                    nc.scalar.activation(out=u_sb[:], in_=pu[:], func=Sig)

                    rh_bf = wp.tile([128, 32], bf16, tag="rh")
                    nc.vector.tensor_tensor(out=rh_bf[:], in0=r_sb[:],
                                            in1=h_prev[:], op=Alu.mult)

                    # h' = ue*h + (1-ue)*c: ue, ue*h, (1-ue) off the critical
                    # chain (need only u), leaving 2 ops after tanh.
                    ue = wp.tile([128, 32], f32, tag="ue")
                    nc.vector.tensor_tensor(out=ue[:], in0=u_sb[:],
                                            in1=invm_sb[:, t * 32:(t + 1) * 32],
                                            op=Alu.max)
                    a_bf = wp.tile([128, 32], bf16, tag="a")
                    nc.vector.tensor_tensor(out=a_bf[:], in0=ue[:],
                                            in1=h_prev[:], op=Alu.mult)
                    w_sb = wp.tile([128, 32], f32, tag="w")
                    nc.vector.tensor_scalar(out=w_sb[:], in0=ue[:],
                                            scalar1=-1.0, scalar2=1.0,
                                            op0=Alu.mult, op1=Alu.add)

                    for ct in range(2):
                        for e in range(2):
                            nc.tensor.matmul(
                                out=pc[:, ct * 16:(ct + 1) * 16],
                                lhsT=wch_sb[:, e, ct * 128:(ct + 1) * 128],
                                rhs=rh_bf[:, e * 16:(e + 1) * 16],
                                start=False, stop=(ct == 1 and e == 1),
                                skip_group_check=True)
                    if t + 1 < t_steps:
                        state["nxt"] = preload(t + 1)
                    c_sb = wp.tile([128, 32], f32, tag="c")
                    nc.scalar.activation(out=c_sb[:], in_=pc[:], func=Tanh)

                    wc_bf = wp.tile([128, 32], bf16, tag="wc")
                    nc.vector.tensor_tensor(out=wc_bf[:], in0=w_sb[:],
                                            in1=c_sb[:], op=Alu.mult)
                    h_new = hp.tile([128, 32], bf16, tag="h")
                    nc.vector.tensor_tensor(out=h_new[:], in0=wc_bf[:],
                                            in1=a_bf[:], op=Alu.add)
                    state["wc"], state["a"] = wc_bf, a_bf

                    if t % SLAB == 0:
                        state["ym"] = yp.tile([128, 2, SLAB * 16], f32, tag="ym", name=f"ym{t}")
                    ym = state["ym"]
                    tr = t % SLAB
                    nc.vector.tensor_tensor(
                        out=ym[:, :, tr * 16:(tr + 1) * 16],
                        in0=h_new[:].rearrange("p (k b) -> p k b", k=2),
                        in1=msk_sb[:, t * 32:(t + 1) * 32].rearrange(
                            "p (k b) -> p k b", k=2),
                        op=Alu.mult)
                    if t % SLAB == SLAB - 1:
                        sl = t // SLAB
                        for k in range(2):
                            nc.sync.dma_start(
                                out=yviews[k][:, sl * SLAB:(sl + 1) * SLAB],
                                in_=ym[:, k].rearrange("p (t b) -> p t b",
                                                       t=SLAB))
                    state["h"] = h_new
                    if t + 1 < t_steps:
                        state["pr"], state["pu"], state["pc"] = state["nxt"]

                emit_group(0)
                if ngrp > 1:
                    emit_group(1)
                state["h"] = h0_bf
                state["wc"], state["a"] = h0_bf, h0_bf
                state["pr"], state["pu"], state["pc"] = preload(0)
                for w in range(t_steps // 32):
                    items = group_items(w + 2) if w + 2 < ngrp else []
                    for i, t in enumerate(range(w * 32, (w + 1) * 32)):
                        emit_step(t, items[i] if i < len(items) else None)

                hf_sb = wp.tile([128, 32], f32, tag="hf")
                nc.scalar.copy(out=hf_sb[:], in_=state["h"][:])
                for k in range(2):
                    nc.sync.dma_start(
                        out=hviews[k][:],
                        in_=hf_sb[:, k * 16:(k + 1) * 16])

    nc.compile()
    return nc


_NC_CACHE = {}


def _get_nc(t_steps):
    if t_steps not in _NC_CACHE:
        _NC_CACHE[t_steps] = build_nc(t_steps)
    return _NC_CACHE[t_steps]


def _prep_core_inputs(inputs, input_lengths, embedder, W_g, b_g, W_c, b_c,
                      t_steps):
    """Host-side slicing/layout prep. Returns list of per-core input maps."""
    inputs = np.asarray(inputs)
    input_lengths = np.asarray(input_lengths)
    embedder = np.ascontiguousarray(np.asarray(embedder, dtype=np.float32))
    W_g = np.asarray(W_g, dtype=np.float32)
    W_c = np.asarray(W_c, dtype=np.float32)
    b_g = np.asarray(b_g, dtype=np.float32)
    b_c = np.asarray(b_c, dtype=np.float32)

    def wprep(w):   # [256, N] -> [128, 2, N] bf16 (k-chunk-major rows)
        n = w.shape[1]
        return np.ascontiguousarray(
            w.reshape(2, 128, n).transpose(1, 0, 2)).astype(BF)

    wgx, wgh = wprep(W_g[:E]), wprep(W_g[E:])
    wcx, wch = wprep(W_c[:E]), wprep(W_c[E:])
    bg = np.ascontiguousarray(b_g.reshape(4, 128).T).astype(np.float32)
    bc = np.ascontiguousarray(b_c.reshape(2, 128).T).astype(np.float32)
    idf = np.eye(128, dtype=np.float32)
    idb = np.eye(128, dtype=BF)

    shared = {"emb": embedder, "wgh": wgh, "wgx": wgx, "wch": wch, "wcx": wcx,
              "bg": bg, "bc": bc, "idf": idf, "idb": idb}

    in_maps = []
    for ci in range(NCORES):
        bsl = slice(ci * BS, (ci + 1) * BS)
        idx_flat = np.ascontiguousarray(inputs[:t_steps, bsl]).reshape(-1)
        idx_cols = np.ascontiguousarray(
            idx_flat.reshape(-1, 128).T).astype(np.int32)
        lens = input_lengths[bsl]
        tgrid = np.arange(t_steps)[:, None, None]          # [t, 1, 1]
        invm = (tgrid >= lens[None, None, :]).astype(BF)   # [t, 1, b]
        invm = np.broadcast_to(invm, (t_steps, 2, BS))
        m = (1.0 - invm.astype(np.float32)).astype(BF)
        im = dict(shared)
        im["idx"] = idx_cols
        im["invm"] = np.ascontiguousarray(invm).reshape(1, -1)
        im["msk"] = np.ascontiguousarray(m).reshape(1, -1)
        in_maps.append(im)
    return in_maps


def run_cores(inputs, input_lengths, embedder, W_g, b_g, W_c, b_c,
              t_steps=T, **spmd_kwargs):
    nc = _get_nc(t_steps)
    in_maps = _prep_core_inputs(inputs, input_lengths, embedder, W_g, b_g,
                                W_c, b_c, t_steps)
    res = run_bass_kernel_spmd(nc, in_maps, core_ids=list(range(NCORES)),
                               **spmd_kwargs)
    outputs = np.concatenate([res.results[i]["y"] for i in range(NCORES)],
                             axis=1)
    h_final = np.concatenate([res.results[i]["hfin"] for i in range(NCORES)],
                             axis=0)
    return (outputs, h_final), res


def kernel(inputs, input_lengths, embedder, W_g, b_g, W_c, b_c):
    (outputs, h_final), _ = run_cores(inputs, input_lengths, embedder,
                                      W_g, b_g, W_c, b_c, t_steps=T)
    return outputs, h_final
